# revision 11
# baseline (speedup 1.0000x reference)
"""Distributed k-NN retrieval kernel for Trainium2 (8 NeuronCores).

Problem: given query `key` [128], memory `keys` [1M, 128], `values` [1M, 128]:
  w_r = 1 / (||key - keys_r||^2 + 1e-3)            (all 1M rows)
  top-50 rows by w; output = sum_i (w_i / sum_all(w)) * values[i]   -> [1, 128]

Strategy (sharding_hint): shard keys row-wise across 8 cores. Each core:
  - streams its (host-pre-transposed) keysT shard [128 feat, F rows] from HBM
  - ScalarE: sq = Square(-k + q) = (q - k)^2 in one pass (q as per-partition bias)
  - TensorE: per 128-row tile, matmul(lhsT=sq_tile[128,128], rhs=ones[128,1])
    -> psum[:, t] = d for those 128 rows (feature reduction on the PE array);
    512 tiles accumulate into one PSUM bank [128, 512]
  - VectorE: banks -> SBUF, w = 1/(d+delta), row-sums (partial denominator)
  - GPSIMD topk: exact top-256 values of -d (= smallest d) + flat indices
Host merges 8 x 256 candidates -> exact global top-50, gathers value rows,
normalizes by the summed denominator.
"""

import numpy as np

MAX_LEN = 1_000_000
N_KEY = 128
QUERY_WIDTH = 50
DELTA = np.float32(1e-3)
N_CORES = 8
ROWS_PER_CORE = 125_056  # ceil(1M / 8) rounded up to a multiple of 128
F = 126_976              # padded rows per core: 31 chunks of 4096
CHUNK = 4096             # rows per DMA/ACT chunk
TOPK_MAX_N = 65_408      # topk ISA _n field is u16; largest 128-multiple
BANK = 512               # psum bank columns (one [128,1] d-column per 128-row tile)
TOPK_K = 256             # gpsimd topk granularity (fixed by the instruction)
PAD_VAL = np.float32(1e18)  # pad rows -> d ~ 1.28e38 -> w ~ 0, never in top-k

_NC_CACHE = {}


def _build_nc(rows=F):
    """Build the per-core Bass program (identical on all cores)."""
    from contextlib import ExitStack

    import concourse.bacc as bacc
    import concourse.mybir as mybir
    import concourse.tile as tile

    f32 = mybir.dt.float32
    u32 = mybir.dt.uint32

    assert rows % CHUNK == 0 and rows % 16 == 0
    n16 = rows // 16
    ntiles = rows // 128
    nsplit = 1 if rows <= TOPK_MAX_N else 2
    part = rows // nsplit          # rows per topk call
    pcols = ntiles // nsplit       # dneg columns per topk call
    assert part % 128 == 0 and 50_000 < part <= TOPK_MAX_N

    nc = bacc.Bacc(
        "TRN2",
        target_bir_lowering=False,
        debug=False,
        enable_asserts=False,
        num_devices=N_CORES,
    )
    keyst = nc.dram_tensor("keyst", [N_KEY, rows], f32, kind="ExternalInput")
    qcol = nc.dram_tensor("qcol", [N_KEY, 1], f32, kind="ExternalInput")
    cand = nc.dram_tensor(
        "cand", [16 * nsplit, TOPK_K // 16 * 2], u32, kind="ExternalOutput"
    )
    wsum = nc.dram_tensor("wsum", [N_KEY, 1], f32, kind="ExternalOutput")

    with tile.TileContext(nc) as tc, ExitStack() as ctx:
        constp = ctx.enter_context(tc.tile_pool(name="const", bufs=1))
        ktp = ctx.enter_context(tc.tile_pool(name="kt", bufs=3))
        sqp = ctx.enter_context(tc.tile_pool(name="sq", bufs=3))
        psp = ctx.enter_context(tc.tile_pool(name="ps", bufs=4, space="PSUM"))
        stp = ctx.enter_context(tc.tile_pool(name="stage", bufs=1))

        qs = constp.tile([N_KEY, 1], f32)
        nc.sync.dma_start(qs[:], qcol.ap())
        ones1 = constp.tile([N_KEY, 1], f32)
        nc.vector.memset(ones1[:], 1.0)

        dfull = stp.tile([128, ntiles], f32)  # d, flat: row = c*128 + p
        ps = None

        for c in range(rows // CHUNK):
            kt = ktp.tile([N_KEY, CHUNK], f32)
            nc.sync.dma_start(kt[:], keyst.ap()[:, c * CHUNK : (c + 1) * CHUNK])
            sq = sqp.tile([N_KEY, CHUNK], f32)
            # sq = Square(kt * -1 + q) = (q - k)^2
            nc.scalar.activation(
                sq[:],
                kt[:],
                mybir.ActivationFunctionType.Square,
                bias=qs[:],
                scale=-1.0,
            )
            for t in range(CHUNK // 128):
                gt = c * (CHUNK // 128) + t  # global 128-row tile index
                b, col = gt // BANK, gt % BANK
                bcols = min(BANK, ntiles - b * BANK)
                if col == 0:
                    ps = psp.tile([128, bcols], f32)
                # psum[p, col] = sum_f sq[f, gt*128 + p] = d for row gt*128+p
                nc.tensor.matmul(
                    ps[:, col : col + 1],
                    sq[:, t * 128 : (t + 1) * 128],
                    ones1[:],
                    start=True,
                    stop=True,
                )
                if col == bcols - 1:
                    nc.vector.tensor_copy(
                        dfull[:, b * BANK : b * BANK + bcols], ps[:]
                    )

        # w = 1 / (d + delta); partial row-sums of w for the denominator
        dplus = stp.tile([128, ntiles], f32)
        nc.vector.tensor_scalar(
            dplus[:], dfull[:], float(DELTA), None, mybir.AluOpType.add
        )
        w128 = stp.tile([128, ntiles], f32)
        nc.vector.reciprocal(w128[:], dplus[:])
        wcol = constp.tile([N_KEY, 1], f32)
        nc.vector.reduce_sum(wcol[:], w128[:], axis=mybir.AxisListType.X)
        nc.sync.dma_start(wsum.ap(), wcol[:])

        # -d for the topk (largest -d = smallest d), reshaped to [16, rows/16].
        # The topk ISA _n field is u16, so large row counts run as two calls
        # over halves of the flat buffer.
        import concourse.bass_isa as bass_isa

        dneg = stp.tile([128, ntiles], f32)
        nc.vector.tensor_scalar(
            dneg[:], dfull[:], -1.0, None, mybir.AluOpType.mult
        )
        d16 = stp.tile([16, n16], f32)
        for s in range(nsplit):
            p16 = part // 16
            nc.sync.dma_start(
                d16[:, s * p16 : (s + 1) * p16],
                dneg[:, s * pcols : (s + 1) * pcols],
            )
            cand_sb = stp.tile(
                [16, TOPK_K // 16 * 2], u32, name=f"cand_sb{s}"
            )
            # nc.gpsimd.topk() but without its SBTensorHandle-only assert
            # (tile-pool tiles are SymbolicTensorHandles; lowering handles them)
            nc.gpsimd.add_instruction(
                bass_isa.InstTopk(
                    name=f"I-{nc.next_id()}",
                    ins=[
                        nc.gpsimd.lower_ap(
                            d16[:, s * p16 : (s + 1) * p16], for_isa=True
                        )
                    ],
                    outs=[nc.gpsimd.lower_ap(cand_sb[:], for_isa=True)],
                    _tokens=1,
                    _n=part,
                    _k=TOPK_K,
                )
            )
            nc.sync.dma_start(cand.ap()[s * 16 : (s + 1) * 16, :], cand_sb[:])

    nc.compile()
    return nc


def _get_nc(rows=F):
    if rows not in _NC_CACHE:
        _NC_CACHE[rows] = _build_nc(rows)
    return _NC_CACHE[rows]


def _make_shards(key, keys):
    """Host-side: transpose + pad keys into per-core [128, F] shards."""
    qcol = np.ascontiguousarray(key.astype(np.float32).reshape(N_KEY, 1))
    in_maps = []
    for c in range(N_CORES):
        base = c * ROWS_PER_CORE
        n_c = max(0, min(ROWS_PER_CORE, MAX_LEN - base))
        sh = np.full((N_KEY, F), PAD_VAL, dtype=np.float32)
        sh[:, :n_c] = keys[base : base + n_c].T
        in_maps.append({"keyst": sh, "qcol": qcol})
    return in_maps


def _rows_from_flat(v, pcols, s):
    """Invert the device flat layout for topk split s.

    Split s's input is dneg[:, s*pcols:(s+1)*pcols] flattened partition-major:
    v = p*pcols + c', and dneg[p, s*pcols + c'] holds row (s*pcols + c')*128 + p.
    """
    p = v // pcols
    c = s * pcols + v % pcols
    return c * 128 + p


def _merge(results, key, keys, values, rows=F):
    """Host-side: merge per-core candidates into the final [1, 128] output."""
    ntiles = rows // 128
    nsplit = 1 if rows <= TOPK_MAX_N else 2
    pcols = ntiles // nsplit
    wsum_parts = [r["wsum"].astype(np.float32).ravel() for r in results]
    W = np.sum(np.concatenate(wsum_parts), dtype=np.float32)

    all_w = []
    all_rows = []
    for c, r in enumerate(results):
        base = c * ROWS_PER_CORE
        n_c = max(0, min(ROWS_PER_CORE, MAX_LEN - base))
        u = np.ascontiguousarray(r["cand"])  # [16*nsplit, 32] uint32
        for s in range(nsplit):
            blk = u[s * 16 : (s + 1) * 16]
            negd = blk[:, : TOPK_K // 16].copy().view(np.float32).ravel()
            flat = blk[:, TOPK_K // 16 :].astype(np.int64).ravel()
            row_local = _rows_from_flat(flat, pcols, s)
            valid = row_local < n_c
            d = -negd[valid]
            all_w.append((np.float32(1.0) / (d + DELTA)).astype(np.float32))
            all_rows.append(base + row_local[valid])
    w = np.concatenate(all_w)
    rows_g = np.concatenate(all_rows)

    # exact top-50 by weight; ties broken by lowest index (lax.top_k behavior)
    order = np.lexsort((rows_g, -w))[:QUERY_WIDTH]
    w50 = w[order]
    rows50 = rows_g[order]
    weights = (w50 / W).astype(np.float32)
    out = np.sum(
        values[rows50].astype(np.float32) * weights[:, None],
        axis=0,
        keepdims=True,
        dtype=np.float32,
    )
    return out.astype(np.float32)


def kernel(key, keys, values, _collect_perf=None):
    """Full-input, full-output entry point. Shards across 8 NeuronCores."""
    from concourse.bass_utils import run_bass_kernel_spmd

    nc = _get_nc()
    in_maps = _make_shards(np.asarray(key), np.asarray(keys))
    res = run_bass_kernel_spmd(
        nc,
        in_maps,
        core_ids=list(range(N_CORES)),
        trace=False,
    )
    if _collect_perf is not None:
        _collect_perf["results"] = res
    return _merge(res.results, np.asarray(key), np.asarray(keys), np.asarray(values))
